# revision 7
# baseline (speedup 1.0000x reference)
"""Multi-head attention (B=2, N=2048, C=1024, H=16, D=64) on 8 TRN2 cores.

Sharding: tensor-parallel over heads — 2 heads per core. Each core computes
Q/K/V projections for its 2 heads, attention, and a partial output
projection (its heads' slice of Wo). Host sums the 8 partial outputs + bo.

v2 structure (vs v1): the attention inner loop is a conveyor paced by
ScalarE exp (~1147ns per key tile). ctx matmuls trail their exp by one
2-key-tile group so the PE FIFO never waits on ACT; projection chains,
V transposes and out-projection matmuls are spread as single-op fillers
between groups instead of bursts. x streams in token-major [128,512]
tiles so the first K/V chain starts ~3us in. Normalize reads ctx straight
from PSUM (no staging copies); out-proj results are copied to bf16 and
DMAd as bf16 partials (host accumulates in fp32).

Per-core dataflow (all matmul inputs bf16, PSUM accumulation fp32):
  xT [1024, 4096] (x transposed on host, replicated to all cores)
  QT/KT = W.T @ x.T   -> [128 (2 heads x 64), 4096]  (lhsT=W chunk, rhs=xT)
  VT likewise, then PE-transposed into v_aug [keys, 65] per head
  (65th column = ones -> softmax denominator comes out of the ctx matmul)
  S^T = K @ Q.T  -> [keys, q] in PSUM (row-tiled: both heads concurrent);
  exp on ScalarE -> bf16 SBUF
  ctx^T_aug [65, q] = v_aug.T @ expS^T  (row 64 = denominator)
  normalize: recip(row 64) on DVE, gpsimd partition_broadcast, DVE multiply
  out_partial [4096, 1024] = ctx^T.T @ Wo_slice  (bf16 out, summed on host)

The 1/sqrt(D) scale is folded into Wq/bq on the host (exact: 0.125).
"""

import numpy as np
import ml_dtypes

import concourse.bass as bass
from concourse import bacc
import concourse.tile as tile
from concourse import mybir, library_config
from concourse.bass_utils import run_bass_kernel_spmd

BF16 = mybir.dt.bfloat16
F32 = mybir.dt.float32

B, N, C = 2, 2048, 1024
H, D = 16, 64
T = B * N              # 4096 tokens
HPC = H // 8           # heads per core = 2
DPC = HPC * D          # head dims per core = 128
KCH = C // 128         # 8 contraction chunks for projections
NCH = T // 512         # 8 token chunks of 512
KT16 = N // 128        # 16 key tiles per batch


def build_core_program(nc):
    xT = nc.dram_tensor("xT", [C, T], BF16, kind="ExternalInput").ap()
    wq = nc.dram_tensor("wq", [C, DPC], BF16, kind="ExternalInput").ap()
    wk = nc.dram_tensor("wk", [C, DPC], BF16, kind="ExternalInput").ap()
    wv = nc.dram_tensor("wv", [C, DPC], BF16, kind="ExternalInput").ap()
    wo = nc.dram_tensor("wo", [DPC, C], BF16, kind="ExternalInput").ap()
    bqkv = nc.dram_tensor("bqkv", [DPC, 3], F32, kind="ExternalInput").ap()
    iden = nc.dram_tensor("iden", [128, 128], BF16, kind="ExternalInput").ap()
    out = nc.dram_tensor("out", [T, C], BF16, kind="ExternalOutput").ap()

    with tile.TileContext(nc) as tc:
        with tc.tile_pool(name="singles", bufs=1) as singles:
            nc.gpsimd.load_library(library_config.proxy)

            id_sb = singles.tile([128, 128], BF16, tag="iden")
            nc.sync.dma_start(out=id_sb, in_=iden)
            bqkv_sb = singles.tile([DPC, 3], F32, tag="bqkv")
            nc.sync.dma_start(out=bqkv_sb, in_=bqkv)
            b_sb = {"q": bqkv_sb[:, 0:1], "k": bqkv_sb[:, 1:2],
                    "v": bqkv_sb[:, 2:3]}

            w_sb = {}
            for nm, w in (("wv", wv), ("wk", wk), ("wq", wq)):
                t = singles.tile([128, KCH, DPC], BF16, tag=nm, name=nm)
                nc.sync.dma_start(
                    out=t, in_=w.rearrange("(k p) j -> p k j", p=128))
                w_sb[nm] = [t[:, k, :] for k in range(KCH)]
            wo_sb = singles.tile([DPC, C], BF16, tag="wo")
            nc.sync.dma_start(out=wo_sb, in_=wo)

            # x resident as 8x8 token-major tiles [128 feat, 512 tok];
            # n-major DMA order so the first K/V chain starts after ~1MB.
            xt = [[singles.tile([128, 512], BF16, tag=f"xt{k}_{n}",
                                name=f"xt{k}_{n}")
                   for n in range(NCH)] for k in range(KCH)]
            for n in range(NCH):
                for k in range(KCH):
                    nc.sync.dma_start(
                        out=xt[k][n],
                        in_=xT[k * 128:(k + 1) * 128, n * 512:(n + 1) * 512])

            QT = singles.tile([128, T], BF16, tag="QT")
            KTt = singles.tile([128, T], BF16, tag="KT")
            VT = singles.tile([128, T], BF16, tag="VT")
            ctxTn = singles.tile([128, T], BF16, tag="ctxTn")
            vaug = [[singles.tile([128, KT16, D + 1], BF16,
                                  tag=f"vaug{b}{h}", name=f"vaug{b}{h}")
                     for h in range(HPC)] for b in range(B)]
            for b in range(B):
                for h in range(HPC):
                    nc.vector.memset(vaug[b][h], 1.0)

            with tc.tile_pool(name="psP", bufs=1, space="PSUM") as psP, \
                    tc.tile_pool(name="psO", bufs=1, space="PSUM") as psO, \
                    tc.tile_pool(name="psS", bufs=2, space="PSUM") as psS, \
                    tc.tile_pool(name="psC", bufs=1, space="PSUM") as psC, \
                    tc.tile_pool(name="esb", bufs=6) as esb, \
                    tc.tile_pool(name="nrm", bufs=3) as nrm, \
                    tc.tile_pool(name="osb", bufs=3) as osb:

                # HAM warmup while weights/x stream in
                for wu in range(12):
                    ptw = psO.tile([128, 128], BF16, tag="po", name="ptw")
                    nc.tensor.transpose(ptw, id_sb, id_sb)

                # ---- filler op factories (each closure emits ~1 PE op) --

                def chain_ops(nm, dstT, nch, act_bias=False):
                    """QKV projection chain: 8 accumulating matmuls into a
                    psP bank + one bias-add move to SBUF. Returns 9 ops."""
                    st = {}

                    def mk(k):
                        def op():
                            if k == 0:
                                st["ps"] = psP.tile([128, 512], F32,
                                                    tag="pj", name="pj")
                            nc.tensor.matmul(
                                out=st["ps"], lhsT=w_sb[nm][k],
                                rhs=xt[k][nch],
                                start=(k == 0), stop=(k == KCH - 1))
                        return op

                    def mv():
                        dst = dstT[:, nch * 512:(nch + 1) * 512]
                        if act_bias:
                            nc.scalar.activation(
                                out=dst, in_=st["ps"],
                                func=mybir.ActivationFunctionType.Identity,
                                bias=b_sb[nm[1]], scale=1.0)
                        else:
                            nc.vector.tensor_scalar_add(
                                out=dst, in0=st["ps"], scalar1=b_sb[nm[1]])
                    return [mk(k) for k in range(KCH)] + [mv]

                def transpose_ops(nch):
                    """4 V transposes for token chunk nch -> vaug tiles."""
                    ops = []

                    def mk(t16):
                        def op():
                            b, bt = divmod(t16, KT16)
                            pt = psO.tile([128, 128], BF16, tag="po",
                                          name="pt")
                            base = t16 * 128
                            nc.tensor.transpose(
                                pt, VT[:, base:base + 128], id_sb)
                            nc.vector.tensor_copy(
                                out=vaug[b][0][:, bt, 0:D], in_=pt[:, 0:D])
                            nc.vector.tensor_copy(
                                out=vaug[b][1][:, bt, 0:D],
                                in_=pt[:, D:2 * D])
                        return op
                    for t16 in range(nch * 4, nch * 4 + 4):
                        ops.append(mk(t16))
                    return ops

                def outproj_ops(ch):
                    """8 out-proj matmuls for chunk ch: MM -> bf16 copy ->
                    DMA [128,512] per op."""
                    q0 = ch * 512
                    ops = []

                    def mk(j):
                        def op():
                            t4, nch2 = divmod(j, 2)
                            tok = q0 + t4 * 128
                            po = psO.tile([128, 512], F32, tag="po",
                                          name="po")
                            nc.tensor.matmul(
                                out=po, lhsT=ctxTn[:, tok:tok + 128],
                                rhs=wo_sb[:, nch2 * 512:(nch2 + 1) * 512],
                                start=True, stop=True)
                            ot = osb.tile([128, 512], BF16, tag="ot",
                                          name="ot")
                            nc.vector.tensor_copy(out=ot, in_=po)
                            nc.sync.dma_start(
                                out=out[tok:tok + 128,
                                        nch2 * 512:(nch2 + 1) * 512],
                                in_=ot)
                        return op
                    for j in range(8):
                        ops.append(mk(j))
                    return ops

                # ---- attention pieces ----------------------------------

                def emit_scores_exp(b, qch, kc):
                    """Row-tiled packed scores (both heads concurrent) +
                    exp. Returns the eS tile."""
                    q0 = b * N + qch * 512
                    k0 = b * N + kc * 128
                    pS = psS.tile([128, 1024], F32, tag="s", name="s")
                    for h in range(HPC):
                        nc.tensor.matmul(
                            out=pS[:, h * 512:(h + 1) * 512],
                            lhsT=KTt[h * D:(h + 1) * D, k0:k0 + 128],
                            rhs=QT[h * D:(h + 1) * D, q0:q0 + 512],
                            start=True, stop=True)
                    eS = esb.tile([128, 1024], BF16, tag="e", name="e")
                    nc.scalar.activation(
                        eS, pS, mybir.ActivationFunctionType.Exp)
                    return eS

                def emit_ctx(b, kc, eS, ctx):
                    for h in range(HPC):
                        nc.tensor.matmul(
                            out=ctx[h],
                            lhsT=vaug[b][h][:, kc, :],
                            rhs=eS[:, h * 512:(h + 1) * 512],
                            start=(kc == 0), stop=(kc == KT16 - 1))

                def emit_normalize(q0, ctx):
                    """denominator recip -> broadcast -> multiply into
                    ctxTn. dn/ctxs staged in SBUF (custom-DVE recip can't
                    read PSUM); the big multiply runs h0 on gpsimd, h1 on
                    DVE as in v1."""
                    bcs, ctxss = [], []
                    for h in range(HPC):
                        dn = nrm.tile([1, 512], F32, tag=f"dn{h}",
                                      name=f"dn{h}")
                        nc.vector.tensor_copy(dn, ctx[h][D:D + 1, :])
                        ctxs = nrm.tile([D, 512], F32, tag=f"ctxs{h}",
                                        name=f"ctxs{h}")
                        nc.vector.tensor_copy(ctxs, ctx[h][0:D, :])
                        rc = nrm.tile([1, 512], F32, tag=f"rc{h}",
                                      name=f"rc{h}")
                        nc.vector.reciprocal_approx_fast(rc, dn)
                        bc = nrm.tile([D, 512], F32, tag=f"bc{h}",
                                      name=f"bc{h}")
                        nc.gpsimd.partition_broadcast(bc, rc)
                        bcs.append(bc)
                        ctxss.append(ctxs)
                    for h in range(HPC):
                        eng = nc.gpsimd if h == 0 else nc.vector
                        eng.tensor_mul(
                            out=ctxTn[h * D:(h + 1) * D, q0:q0 + 512],
                            in0=ctxss[h], in1=bcs[h])

                # ---- schedule ------------------------------------------

                # pre-conveyor: V/K/Q chains for token chunk 0 (ACT moves —
                # ScalarE is idle before the exp conveyor starts).
                for op in chain_ops("wv", VT, 0, act_bias=True):
                    op()
                t_pre = transpose_ops(0)
                wk0 = chain_ops("wk", KTt, 0, act_bias=True)
                for i, op in enumerate(wk0):
                    op()
                    if i % 3 == 2 and t_pre:
                        t_pre.pop(0)()
                for op in t_pre:
                    op()
                for op in chain_ops("wq", QT, 0, act_bias=True):
                    op()

                # per-chunk filler queues (chunks 0-7 in token order;
                # chunk i covers tokens i*512..i*512+511). Each entry is
                # (ops, marks): marks[label] = index in ops after which
                # that chain/transpose set is fully EMITTED — used for
                # deadline pumping so a consumer is never emitted before
                # its producer (Tile deps follow program order).
                def build(parts):
                    ops, marks = [], {}
                    for label, lops in parts:
                        ops.extend(lops)
                        if label:
                            marks[label] = len(ops)
                    return [ops, marks]

                fills = [None] * 8
                fills[0] = build([
                    ("wk1", chain_ops("wk", KTt, 1)),
                    ("wv1", chain_ops("wv", VT, 1)),
                    ("T1", transpose_ops(1)),
                    ("wk2", chain_ops("wk", KTt, 2)),
                    ("wv2", chain_ops("wv", VT, 2)),
                    ("T2", transpose_ops(2)),
                    ("wk3", chain_ops("wk", KTt, 3)),
                    ("wv3", chain_ops("wv", VT, 3)),
                    ("T3", transpose_ops(3)),
                    (None, chain_ops("wq", QT, 1)),
                ])
                fills[1] = build([
                    (None, chain_ops("wk", KTt, 4)),
                    (None, chain_ops("wv", VT, 4)),
                    (None, transpose_ops(4)),
                    (None, chain_ops("wq", QT, 2)),
                ])
                fills[2] = build([
                    (None, chain_ops("wk", KTt, 5)),
                    (None, chain_ops("wv", VT, 5)),
                    (None, transpose_ops(5)),
                    (None, chain_ops("wk", KTt, 6)),
                    (None, chain_ops("wq", QT, 3)),
                ])
                fills[3] = build([
                    (None, chain_ops("wv", VT, 6)),
                    (None, transpose_ops(6)),
                    (None, chain_ops("wk", KTt, 7)),
                    (None, chain_ops("wv", VT, 7)),
                    (None, transpose_ops(7)),
                    (None, chain_ops("wq", QT, 4)),
                ])
                fills[4] = build([(None, chain_ops("wq", QT, 5))])
                fills[5] = build([(None, chain_ops("wq", QT, 6))])
                fills[6] = build([(None, chain_ops("wq", QT, 7))])
                fills[7] = build([])

                for ch in range(8):
                    b, qch = divmod(ch, 4)
                    q0 = ch * 512
                    fq, marks = fills[ch]
                    pumped = [0]

                    def pump(n):
                        for _ in range(min(n, len(fq))):
                            fq.pop(0)()
                            pumped[0] += 1

                    def pump_to(idx):
                        if idx is not None:
                            pump(idx - pumped[0])

                    ctx = [psC.tile([D + 1, 512], F32, tag=f"ctx{h}",
                                    name=f"ctx{h}") for h in range(HPC)]
                    eS_pend = []  # (kc, eS) waiting for their ctx
                    for g in range(8):
                        for kc in (2 * g, 2 * g + 1):
                            pump_to(marks.get(f"wk{kc // 4}"))
                            eS_pend.append(
                                (kc, emit_scores_exp(b, qch, kc)))
                        # ctx trails by one group: drain all but the
                        # newest 2 eS tiles
                        while len(eS_pend) > 2:
                            kc0, e0 = eS_pend.pop(0)
                            pump_to(marks.get(f"T{kc0 // 4}"))
                            emit_ctx(b, kc0, e0, ctx)
                        pump(-(-len(fq) // (8 - g)))
                    while eS_pend:
                        kc0, e0 = eS_pend.pop(0)
                        pump_to(marks.get(f"T{kc0 // 4}"))
                        emit_ctx(b, kc0, e0, ctx)
                    pump(len(fq))
                    emit_normalize(q0, ctx)
                    if ch + 1 < 8:
                        fills[ch + 1][0].extend(outproj_ops(ch))
                    else:
                        for op in outproj_ops(ch):
                            op()
    return nc


_NC_CACHE = None


def _get_nc():
    global _NC_CACHE
    if _NC_CACHE is None:
        nc = bacc.Bacc("TRN2", target_bir_lowering=False)
        build_core_program(nc)
        nc.finalize()
        _NC_CACHE = nc
    return _NC_CACHE


def make_in_maps(x, Wq, bq, Wk, bk, Wv, bv, Wo):
    bf = ml_dtypes.bfloat16
    x = np.asarray(x, np.float32).reshape(T, C)
    xT_bf = np.ascontiguousarray(x.T).astype(bf)
    iden = np.eye(128, dtype=bf)
    Wq = np.asarray(Wq, np.float32)
    Wk = np.asarray(Wk, np.float32)
    Wv = np.asarray(Wv, np.float32)
    Wo = np.asarray(Wo, np.float32)
    bq = np.asarray(bq, np.float32)
    bk = np.asarray(bk, np.float32)
    bv = np.asarray(bv, np.float32)
    in_maps = []
    for cidx in range(8):
        hs = slice(cidx * DPC, (cidx + 1) * DPC)
        in_maps.append(dict(
            xT=xT_bf,
            wq=np.ascontiguousarray(Wq[:, hs] * 0.125).astype(bf),
            wk=np.ascontiguousarray(Wk[:, hs]).astype(bf),
            wv=np.ascontiguousarray(Wv[:, hs]).astype(bf),
            wo=np.ascontiguousarray(Wo[hs, :]).astype(bf),
            bqkv=np.stack([bq[hs] * 0.125, bk[hs], bv[hs]],
                          axis=1).astype(np.float32),
            iden=iden,
        ))
    return in_maps


def kernel(x, Wq, bq, Wk, bk, Wv, bv, Wo, bo, _trace=False, _trace_kwargs=None):
    in_maps = make_in_maps(x, Wq, bq, Wk, bk, Wv, bv, Wo)
    nc = _get_nc()
    res = run_bass_kernel_spmd(
        nc, in_maps, core_ids=list(range(8)),
        trace=_trace, **(_trace_kwargs or {}))
    acc = res.results[0]["out"].astype(np.float32)
    for cidx in range(1, 8):
        acc += res.results[cidx]["out"].astype(np.float32)
    acc += np.asarray(bo, np.float32)[None, :]
    out = acc.reshape(B, N, C)
    kernel.last_results = res
    return out
